# revision 1
# baseline (speedup 1.0000x reference)
"""Trainium2 Bass kernel for the EntangledInterferenceLayer problem.

Math transformations done on host (numpy), all exact up to fp rounding:
  * The HxH entanglement mix commutes with RoPE (cos/sin are head-independent),
    so it folds into the Q/K projection weights + biases:
        mix(rope(x @ W + b)) = rope(x @ (W folded with ent) + (b folded)).
  * The per-head phase shift rotates q and k by the same complex phase, and the
    attention logits use q * conj(k) -> the phase cancels exactly.  Dropped.
  * 1/sqrt(head_dim) folds into the Q weights/bias.
  * The V-projection bias contributes bv @ Wo to every output row (softmax rows
    sum to 1), so it folds into the output bias.

Sharding (8 cores): core = (batch b, head-group g of 4 heads). Each core
projects Q/K/V for its heads, runs causal complex-magnitude attention, then an
AllGather of attention outputs within the 4-core batch group lets every core
compute a 256-column slice of both output projections.

Device layout notes:
  * Q/K are computed directly transposed ([head-dim, token]) by using W as the
    stationary matmul operand and x^T as the moving one.
  * Per head the 128 contraction rows are [qr-rot(32), qr-nonrot(32),
    qi-rot(32), qi-nonrot(32)]; K2 = [-ki-rot, -ki-nr, kr-rot, kr-nr] gives the
    imaginary logits with plain matmuls.
  * Scores are built [kv, q] so softmax-normalisation sums arrive via a ones
    column appended to V, and the attention output lands pre-transposed for the
    output projection.
"""

import math

import numpy as np

B, S, DIM = 2, 1024, 1024
HEADS, HD, ROTD = 16, 64, 32
GH = 4  # heads per core
ODC = 256  # out-dim columns per core
NCORES = 8

_PAIRSWAP = [i ^ 1 for i in range(32)]

# 'float32' (exact, 4 cyc/row) or 'float32r' (full-rate, relaxed precision)
MM_DTYPE = "float32r"


def _register_magsq():
    """Register a fused custom DVE op: out = (in0^2 + in1^2) * imm2."""
    import numpy as np
    from concourse import dve_ops as DO
    from concourse.dve_spec import Spec, Src0, Src1, C2, sq, lower

    if "ANT_MAGSQ" in DO._SUB_OPCODE_FOR_NAME:
        return next(o for o in DO.OPS if o.name == "ANT_MAGSQ")
    spec = Spec(
        body=(sq(Src0) + sq(Src1)) * C2,
        reference=lambda in0, in1, s0, s1, imm2: (
            in0.astype(np.float32) ** 2 + in1.astype(np.float32) ** 2
        )
        * np.float32(imm2),
    )
    opcode = DO._CUSTOM_DVE_ROW_BASE + len(DO.OPS)
    DO._SUB_OPCODE_FOR_NAME["ANT_MAGSQ"] = opcode
    shas = {}
    for ver in ("v3", "v4"):
        try:
            s = DO.DveOpSpec(
                name="ANT_MAGSQ", opcode=opcode, uops=lower(spec, ver=ver), rd1_en=True
            )
            shas[ver] = s.sha(ver)
        except Exception:
            pass
    op = DO.DveOp("ANT_MAGSQ", spec, subdim=False, uops_sha=shas)
    DO.OPS.append(op)
    DO.CUSTOM_DVE_SPECS["ANT_MAGSQ"] = spec
    return op


def _build(gt: float, groups=None, mm_dtype=None):
    import concourse.mybir as mybir
    import concourse.tile as tile
    from concourse import bacc

    f32 = mybir.dt.float32
    mdt = getattr(mybir.dt, mm_dtype or MM_DTYPE)
    AF = mybir.ActivationFunctionType
    magsq = _register_magsq()

    nc = bacc.Bacc("TRN2", target_bir_lowering=False, num_devices=NCORES)
    if groups is None:
        groups = [[0, 1, 2, 3], [4, 5, 6, 7]]

    xr = nc.dram_tensor("xr_t", [DIM, S], mdt, kind="ExternalInput")
    xi = nc.dram_tensor("xi_t", [DIM, S], mdt, kind="ExternalInput")
    w = {
        nm: nc.dram_tensor(nm, [DIM, 256], mdt, kind="ExternalInput")
        for nm in ["wqr", "wqi", "wkr", "wki", "wvr", "wvi", "wor", "woi"]
    }
    bias_d = {
        nm: nc.dram_tensor(nm, [1, 256], mdt, kind="ExternalInput")
        for nm in ["bqr", "bqi", "bkr", "bki", "bor", "boi"]
    }
    cosd = nc.dram_tensor("cosd", [128, S], f32, kind="ExternalInput")
    sind = nc.dram_tensor("sind", [128, S], f32, kind="ExternalInput")
    o_r = nc.dram_tensor("o_r", [ODC, S], f32, kind="ExternalOutput")
    o_i = nc.dram_tensor("o_i", [ODC, S], f32, kind="ExternalOutput")

    def mm(out, lhsT, rhs, start, stop):
        nc.tensor.matmul(out, lhsT=lhsT, rhs=rhs, start=start, stop=stop)

    with tile.TileContext(nc) as tc:
        with (
            tc.tile_pool(name="consts", bufs=1) as consts,
            tc.tile_pool(name="bigt", bufs=18) as bigt,
            tc.tile_pool(name="wqk", bufs=2) as wqkp,
            tc.tile_pool(name="wv", bufs=1) as wvp,
            tc.tile_pool(name="wo", bufs=1) as wop,
            tc.tile_pool(name="persist", bufs=1) as persist,
            tc.tile_pool(name="stage", bufs=2) as stage,
            tc.tile_pool(name="evp", bufs=2) as evp,
            tc.tile_pool(name="ep_s", bufs=2) as ep_s,
            tc.tile_pool(name="small", bufs=2) as small,
            tc.tile_pool(name="lop", bufs=2) as lop,
            tc.tile_pool(name="ps", bufs=8, space="PSUM") as ps,
            tc.tile_pool(name="dram", bufs=1, space="DRAM") as dram,
        ):
            ones_t = consts.tile([1, 512], mdt, tag="ones")
            nc.vector.memset(ones_t.bitcast(f32), 1.0)
            eps_t = consts.tile([128, 1], f32, tag="eps")
            nc.vector.memset(eps_t, 1e-6 * float(gt) * float(gt))
            cos_sb = consts.tile([128, S], f32, tag="cos")
            nc.sync.dma_start(cos_sb, cosd[:, :])
            sin_sb = consts.tile([128, S], f32, tag="sin")
            nc.sync.dma_start(sin_sb, sind[:, :])
            bias_sb = {}
            for nm in bias_d:
                t = consts.tile([1, 256], mdt, tag=nm)
                nc.sync.dma_start(t, bias_d[nm][:, :])
                bias_sb[nm] = t
            Q = persist.tile([128, GH, S], mdt, tag="Q")
            K1 = persist.tile([128, GH, S], mdt, tag="K1")
            K2 = persist.tile([128, GH, S], mdt, tag="K2")
            Vr = persist.tile([128, 8, GH, 65], mdt, tag="Vr")
            Vi = persist.tile([128, 8, GH, 64], mdt, tag="Vi")
            nc.vector.memset(Vr[:, :, :, 64:65].bitcast(f32), 1.0)

            # (name, x-source key, W, bias, rot-target rows, nr-target rows)
            # targets: list of (tensor, row0); K2 entries negated where noted.
            projs = [
                ("qr", "r", "wqr", "bqr", [(0, 0)], [(0, 32)]),
                ("qi", "i", "wqi", "bqi", [(0, 64)], [(0, 96)]),
                ("kr", "r", "wkr", "bkr", [(1, 0), (2, 64)], [(1, 32), (2, 96)]),
                ("ki", "i", "wki", "bki", [(1, 64)], [(1, 96)]),
            ]
            qk_tensors = {0: Q, 1: K1, 2: K2}

            for c in range(2):
                csl = slice(c * 512, (c + 1) * 512)
                x_t = {"r": [], "i": []}
                for key, src in (("r", xr), ("i", xi)):
                    for kt in range(8):
                        t = bigt.tile([128, 512], mdt, tag="big")
                        nc.sync.dma_start(
                            t, src[kt * 128 : (kt + 1) * 128, csl]
                        )
                        x_t[key].append(t)

                for pname, xkey, wname, bname, rot_tgts, nr_tgts in projs:
                    for mt in range(2):  # 0 = rot dims, 1 = non-rot dims
                        w_sb = wqkp.tile([128, 8, 128], mdt, tag="wqk")
                        nc.sync.dma_start(
                            w_sb,
                            w[wname][:, mt * 128 : (mt + 1) * 128].rearrange(
                                "(kt kp) m -> kp kt m", kp=128
                            ),
                        )
                        pst = ps.tile([128, 512], f32, tag="ps")
                        for kt in range(8):
                            mm(
                                pst,
                                w_sb[:, kt, :],
                                x_t[xkey][kt],
                                start=(kt == 0),
                                stop=False,
                            )
                        # bias via K=1 matmul (bias values as stationary operand)
                        mm(
                            pst,
                            bias_sb[bname][:, mt * 128 : (mt + 1) * 128],
                            ones_t,
                            start=False,
                            stop=True,
                        )
                        if mt == 0:
                            shuf = stage.tile([128, 512], f32, tag="shuf")
                            nc.vector.stream_shuffle(shuf, pst, mask=_PAIRSWAP)
                            nc.vector.tensor_mul(shuf, shuf, sin_sb[:, csl])
                            t2 = stage.tile([128, 512], mdt, tag="t2")
                            nc.vector.tensor_mul(t2, pst, cos_sb[:, csl])
                            nc.vector.tensor_add(t2, t2, shuf)
                            src_t = t2
                        else:
                            evn = evp.tile([128, 512], mdt, tag="ev")
                            nc.scalar.copy(evn, pst)
                            src_t = evn
                        tgts = rot_tgts if mt == 0 else nr_tgts
                        for tid, row0 in tgts:
                            dst = qk_tensors[tid]
                            for h in range(GH):
                                nc.sync.dma_start(
                                    dst[row0 : row0 + 32, h, csl],
                                    src_t[h * 32 : (h + 1) * 32, :],
                                )
                        if pname == "ki":  # negated copy into K2 rows 0:32 / 32:64
                            neg = evp.tile([128, 512], mdt, tag="ev")
                            nc.vector.tensor_scalar_mul(neg, src_t, -1.0)
                            row0 = 0 if mt == 0 else 32
                            for h in range(GH):
                                nc.sync.dma_start(
                                    K2[row0 : row0 + 32, h, csl],
                                    neg[h * 32 : (h + 1) * 32, :],
                                )

                for vname, xkey, Vt, vw in (
                    ("vr", "r", Vr, "wvr"),
                    ("vi", "i", Vi, "wvi"),
                ):
                    wv_sb = wvp.tile([128, 8, 256], mdt, tag="wv", bufs=2)
                    nc.sync.dma_start(
                        wv_sb, w[vw].rearrange("(kt kp) m -> kp kt m", kp=128)
                    )
                    for tl in range(4):
                        tt = c * 4 + tl
                        pv = ps.tile([128, 256], f32, tag="ps")
                        for kt in range(8):
                            mm(
                                pv,
                                x_t[xkey][kt][:, tl * 128 : (tl + 1) * 128],
                                wv_sb[:, kt, :],
                                start=(kt == 0),
                                stop=(kt == 7),
                            )
                        ov = evp.tile([128, 256], mdt, tag="ov")
                        nc.scalar.copy(ov, pv)
                        nc.sync.dma_start(
                            Vt[:, tt, :, 0:64],
                            ov.rearrange("p (h d) -> p h d", h=GH),
                        )

            # ---- attention ----
            # One AllGather buffer per 512-token chunk so the collective for
            # chunk 0 overlaps attention on chunk 1 and the output projection.
            agin = [dram.tile([512, 512], mdt, tag=f"agin{qc}", name=f"agin{qc}") for qc in range(2)]
            agout = [
                dram.tile([4, 512, 512], mdt, tag=f"agout{qc}", name=f"agout{qc}") for qc in range(2)
            ]

            for qc in range(2):
                qcs = slice(qc * 512, (qc + 1) * 512)
                for h in range(GH):
                    qsl = Q[:, h, qcs]
                    nkv = (qc + 1) * 4
                    ets = []
                    # phase 1: scores -> fused |z|^2 for every kv tile
                    for kvt in range(nkv):
                        ksl = slice(kvt * 128, (kvt + 1) * 128)
                        psr = ps.tile([128, 512], f32, tag="ps")
                        mm(psr, K1[:, h, ksl], qsl, start=True, stop=True)
                        psi = ps.tile([128, 512], f32, tag="ps")
                        mm(psi, K2[:, h, ksl], qsl, start=True, stop=True)
                        c1 = ep_s.tile([128, 512], f32, tag="c1")
                        nc.vector.tensor_copy(c1, psi)
                        sq = bigt.tile([128, 512], f32, tag="big")
                        # sq <- (psr^2 + c1^2) * gt^2  (fused custom DVE op)
                        nc.vector._custom_dve(
                            magsq, out=sq, in0=psr, in1=c1,
                            imm2=float(gt) * float(gt),
                        )
                        ets.append(sq)
                    # batched ACT passes (one table load per function)
                    for sq in ets:
                        nc.scalar.activation(sq, sq, AF.Sqrt, bias=eps_t)
                    for kvt in range(nkv):
                        et = bigt.tile([128, 512], mdt, tag="big")
                        nc.scalar.activation(et, ets[kvt], AF.Exp, scale=1.0)
                        ets[kvt] = et
                    for kvt in range(nkv):
                        off = kvt - qc * 4
                        if off >= 0:
                            # causal: keep where q_local - kv_local - 128*off >= 0
                            nc.gpsimd.affine_select(
                                out=ets[kvt],
                                in_=ets[kvt],
                                compare_op=mybir.AluOpType.is_ge,
                                fill=0.0,
                                base=-(off * 128),
                                channel_multiplier=-1,
                                pattern=[[1, 512]],
                            )
                    # phase 2: dense AV accumulation (PSUM held only briefly)
                    avr = ps.tile([65, 512], f32, tag="ps")
                    avi = ps.tile([64, 512], f32, tag="ps")
                    for kvt in range(nkv):
                        mm(avr, Vr[:, kvt, h, :], ets[kvt],
                           start=(kvt == 0), stop=(kvt == nkv - 1))
                        mm(avi, Vi[:, kvt, h, :], ets[kvt],
                           start=(kvt == 0), stop=(kvt == nkv - 1))
                    # Copy sums out of PSUM immediately (frees the banks),
                    # then normalise from SBUF and stage for the AllGather.
                    onr = evp.tile([65, 512], f32, tag="on", bufs=2)
                    nc.scalar.copy(onr, avr)
                    oni = evp.tile([64, 512], f32, tag="oni", bufs=2)
                    nc.scalar.copy(oni, avi)
                    lrec = small.tile([1, 512], f32, tag="lrec")
                    nc.vector.reciprocal(lrec, onr[64:65, :])
                    lrd = dram.tile([1, 512], f32, tag="lrd", bufs=4)
                    nc.sync.dma_start(lrd, lrec)
                    lbc = small.tile([64, 512], f32, tag="lbc", bufs=2)
                    nc.sync.dma_start(lbc, lrd.to_broadcast((64, 512)))
                    onn = evp.tile([64, 512], mdt, tag="onn", bufs=2)
                    nc.vector.tensor_mul(onn, onr[0:64, :], lbc)
                    nc.sync.dma_start(agin[qc][h * 64 : (h + 1) * 64, :], onn)
                    onn2 = evp.tile([64, 512], mdt, tag="onn", bufs=2)
                    nc.vector.tensor_mul(onn2, oni, lbc)
                    nc.sync.dma_start(
                        agin[qc][256 + h * 64 : 256 + (h + 1) * 64, :], onn2
                    )

            # ---- per chunk: AllGather, denominators, output projection ----
            # AG(qc+1) is issued while the engines work through O-proj(qc),
            # so only the first gather's latency is exposed.
            wo_sb = {}
            for ri, wname in ((0, "wor"), (1, "woi")):
                t = wop.tile([128, 8, 256], mdt, tag=wname, name=f"wo{ri}")
                nc.sync.dma_start(
                    t, w[wname].rearrange("(kt kp) m -> kp kt m", kp=128)
                )
                wo_sb[ri] = t

            for qc in range(2):
                nc.gpsimd.collective_compute(
                    "AllGather",
                    mybir.AluOpType.bypass,
                    replica_groups=groups,
                    ins=[agin[qc][:].opt()],
                    outs=[agout[qc][:].opt()],
                )
                # out.T[od, tok] = Wo.T @ normalised-attn-out
                for ri, bname, odst in ((0, "bor", o_r), (1, "boi", o_i)):
                    pos = [
                        ps.tile([128, 512], f32, tag="ps", name=f"po{_i}")
                        for _i in range(2)
                    ]
                    for ht in range(8):
                        g, half = ht // 2, ht % 2
                        r0 = ri * 256 + half * 128
                        lt = lop.tile([128, 512], mdt, tag="lt", bufs=4)
                        nc.sync.dma_start(lt, agout[qc][g, r0 : r0 + 128, :])
                        for odt in range(2):
                            mm(pos[odt], wo_sb[ri][:, ht, odt * 128 : (odt + 1) * 128],
                               lt, start=(ht == 0), stop=False)
                    for odt in range(2):
                        mm(pos[odt], bias_sb[bname][:, odt * 128 : (odt + 1) * 128],
                           ones_t, start=False, stop=True)
                        oo = evp.tile([128, 512], f32, tag="oo")
                        nc.scalar.copy(oo, pos[odt])
                        nc.sync.dma_start(
                            odst[odt * 128 : (odt + 1) * 128,
                                 qc * 512 : (qc + 1) * 512],
                            oo,
                        )

    return nc


def _host_prep(inputs):
    """Fold ent/scale/bv on host; build per-core input maps."""
    f = lambda x: np.asarray(x, dtype=np.float32)
    real, imag = f(inputs["real"]), f(inputs["imag"])
    ent = np.asarray(inputs["ent"], np.float64)
    scale = 1.0 / math.sqrt(HD)

    def fold_w(W, do_ent, sc=1.0):
        W = np.asarray(W, np.float64).reshape(DIM, HEADS, HD)
        if do_ent:
            W = np.einsum("chd,hx->cxd", W, ent)
        return W * sc  # [DIM, HEADS, HD] float64

    def fold_b(b, do_ent, sc=1.0):
        b = np.asarray(b, np.float64).reshape(HEADS, HD)
        if do_ent:
            b = np.einsum("hd,hx->xd", b, ent)
        return b * sc

    Wq_r = fold_w(inputs["Wq_r"], True, scale)
    Wq_i = fold_w(inputs["Wq_i"], True, scale)
    Wk_r = fold_w(inputs["Wk_r"], True)
    Wk_i = fold_w(inputs["Wk_i"], True)
    Wv_r = fold_w(inputs["Wv_r"], False)
    Wv_i = fold_w(inputs["Wv_i"], False)
    bq_r = fold_b(inputs["bq_r"], True, scale)
    bq_i = fold_b(inputs["bq_i"], True, scale)
    bk_r = fold_b(inputs["bk_r"], True)
    bk_i = fold_b(inputs["bk_i"], True)
    Wo_r = np.asarray(inputs["Wo_r"], np.float64)
    Wo_i = np.asarray(inputs["Wo_i"], np.float64)
    bo_r = np.asarray(inputs["bo_r"], np.float64) + np.asarray(
        inputs["bv_r"], np.float64
    ) @ Wo_r
    bo_i = np.asarray(inputs["bo_i"], np.float64) + np.asarray(
        inputs["bv_i"], np.float64
    ) @ Wo_i

    strength = float(np.asarray(inputs["strength"]).reshape(-1)[0])
    temp = float(np.asarray(inputs["temp"]).reshape(-1)[0])
    gt = (1.0 / (1.0 + math.exp(-strength))) / max(temp, 0.01)

    # rope tables in device layout: row h*32+d (d<32), freq j=d//2
    rot_freqs = np.asarray(inputs["rot_freqs"], np.float64)  # [16]
    pos = np.arange(S, dtype=np.float64)
    emb = pos[:, None] * rot_freqs[None, :]  # [S, 16]
    cos_t = np.cos(emb)  # [S,16]
    sin_t = np.sin(emb)
    cosd = np.empty((128, S), np.float32)
    sind = np.empty((128, S), np.float32)
    for hh in range(4):
        for d in range(32):
            r = hh * 32 + d
            cosd[r] = cos_t[:, d // 2]
            sind[r] = (-sin_t if d % 2 == 0 else sin_t)[:, d // 2]

    def qk_dev(Wf, bf, g):
        # [DIM,H,HD]/[H,HD] -> per-core [DIM,256]/[1,256] in [rot x 4h | nr x 4h]
        hs = slice(g * GH, (g + 1) * GH)
        Wc, bc = Wf[:, hs, :], bf[hs, :]
        wd = np.concatenate(
            [
                Wc[:, :, :ROTD].reshape(DIM, GH * ROTD),
                Wc[:, :, ROTD:].reshape(DIM, GH * ROTD),
            ],
            axis=1,
        )
        bd = np.concatenate(
            [bc[:, :ROTD].reshape(1, GH * ROTD), bc[:, ROTD:].reshape(1, GH * ROTD)],
            axis=1,
        )
        return wd.astype(np.float32), bd.astype(np.float32)

    in_maps = []
    for core in range(NCORES):
        b, g = core // 4, core % 4
        hs = slice(g * GH, (g + 1) * GH)
        m = {
            "xr_t": np.ascontiguousarray(real[b].T),
            "xi_t": np.ascontiguousarray(imag[b].T),
            "cosd": cosd,
            "sind": sind,
            "wvr": np.ascontiguousarray(Wv_r[:, hs, :].reshape(DIM, 256)).astype(
                np.float32
            ),
            "wvi": np.ascontiguousarray(Wv_i[:, hs, :].reshape(DIM, 256)).astype(
                np.float32
            ),
            "wor": np.ascontiguousarray(Wo_r[:, g * ODC : (g + 1) * ODC]).astype(
                np.float32
            ),
            "woi": np.ascontiguousarray(Wo_i[:, g * ODC : (g + 1) * ODC]).astype(
                np.float32
            ),
            "bor": bo_r[None, g * ODC : (g + 1) * ODC].astype(np.float32),
            "boi": bo_i[None, g * ODC : (g + 1) * ODC].astype(np.float32),
        }
        for nm, Wf, bf in (
            ("qr", Wq_r, bq_r),
            ("qi", Wq_i, bq_i),
            ("kr", Wk_r, bk_r),
            ("ki", Wk_i, bk_i),
        ):
            wd, bd = qk_dev(Wf, bf, g)
            m["w" + nm] = wd
            m["b" + nm] = bd
        in_maps.append(m)
    return in_maps, gt


def kernel(**inputs):
    from concourse import bass_utils

    in_maps, gt = _host_prep(inputs)
    nc = _build(gt)
    nc.finalize()
    res = bass_utils.run_bass_kernel_spmd(
        nc, in_maps, core_ids=list(range(NCORES))
    )
    out_r = np.empty((B, S, DIM), np.float32)
    out_i = np.empty((B, S, DIM), np.float32)
    for core in range(NCORES):
        b, g = core // 4, core % 4
        out_r[b, :, g * ODC : (g + 1) * ODC] = res.results[core]["o_r"].T
        out_i[b, :, g * ODC : (g + 1) * ODC] = res.results[core]["o_i"].T
    return np.stack([out_r, out_i], axis=0)



# revision 11
# speedup vs baseline: 1.4679x; 1.4679x over previous
"""Trainium2 Bass kernel for the EntangledInterferenceLayer problem (v2).

Math transformations on host (numpy float64, exact up to fp rounding):
  * HxH entanglement mix commutes with RoPE -> folded into Q/K weights+biases.
  * Per-head phase shift cancels in q*conj(k) -> dropped.
  * 1/sqrt(head_dim) folded into Q weights/bias.
  * V-projection bias contributes bv @ Wo to every row -> folded into out bias.

Sharding (8 cores): core = (batch b, head-group g of 4 heads). Device compute
in bf16 (fp32 PSUM/softmax internals); rel tolerance is 2e-2.

v2 design vs the original baseline:
  * Projections computed per head-PAIR: stationary [128, h0|h1 x 64 dims],
    PSUM rows = [h0 rot|nr | h1 rot|nr]; bias added by ScalarE (Identity with
    per-partition bias AP); RoPE applied by DVE (stream_shuffle pairswap +
    cos/sin tables with 1/0 rows for non-rot dims) writing directly into the
    per-head Q/K1 stacks via partition-offset-shifted adds. No bias matmuls,
    no SBUF->SBUF staging DMAs.
  * K2 = [-ki | kr] built with two 64-row DVE copies per head.
  * V written straight from PSUM into the persistent V tile (strided ACT copy).
  * All weights/x pre-packed on host into the exact SBUF layouts (contiguous
    DMA), loaded once.
  * Attention trimmed to the causal support: kv-tile (q >= 128*off) column
    ranges only (~25% less score/softmax/AV work); sqrt/exp batched per
    head-pair so ACT reloads tables only 2x per pair.
  * Softmax denominator: ones-column in Vr; reciprocal via gpsimd
    partition_broadcast + reciprocal_approx_fast; normalized outputs written
    into the local slot of the recv buffer as pure-real/imag 2-head chunks.
  * Comms: remote_dma_broadcast (XOR-relative dests) pushes each pair's
    normalized output [128, 2, 512] bf16 directly into the 3 peers' recv SBUF
    slots; arrival via remote semaphore (+2/send); receive side waits inside
    a tile_critical so the Tile scheduler doesn't deadlock. A tiny AllReduce
    barrier + sem_clear at kernel start makes re-execution of the NEFF safe;
    sends are ordered after the barrier via data edges (barrier result DMA'd
    into each payload chunk corner, overwritten by the normalize muls).
  * Output projection contracts recv chunks (K=128 fully used since chunks
    are pure real/imag) with per-core slot-permuted Wo (host knows each
    core's XOR peer map), + ScalarE bias, f32 out.
"""

import math
import os

import numpy as np

_NO_SEND = os.environ.get("ANT_NO_SEND") == "1"
_NO_WAIT = os.environ.get("ANT_NO_WAIT") == "1"

B, S, DIM = 2, 1024, 1024
HEADS, HD, ROTD = 16, 64, 32
GH = 4  # heads per core
ODC = 256  # out-dim columns per core
NCORES = 8

_PAIRSWAP = [i ^ 1 for i in range(32)]


def _register_magsq():
    """Register a fused custom DVE op: out = (in0^2 + in1^2) * imm2."""
    import numpy as np
    from concourse import dve_ops as DO
    from concourse.dve_spec import Spec, Src0, Src1, C2, sq, lower

    if "ANT_MAGSQ" in DO._SUB_OPCODE_FOR_NAME:
        return next(o for o in DO.OPS if o.name == "ANT_MAGSQ")
    spec = Spec(
        body=(sq(Src0) + sq(Src1)) * C2,
        reference=lambda in0, in1, s0, s1, imm2: (
            in0.astype(np.float32) ** 2 + in1.astype(np.float32) ** 2
        )
        * np.float32(imm2),
    )
    opcode = DO._CUSTOM_DVE_ROW_BASE + len(DO.OPS)
    DO._SUB_OPCODE_FOR_NAME["ANT_MAGSQ"] = opcode
    shas = {}
    for ver in ("v3", "v4"):
        try:
            s = DO.DveOpSpec(
                name="ANT_MAGSQ", opcode=opcode, uops=lower(spec, ver=ver), rd1_en=True
            )
            shas[ver] = s.sha(ver)
        except Exception:
            pass
    op = DO.DveOp("ANT_MAGSQ", spec, subdim=False, uops_sha=shas)
    DO.OPS.append(op)
    DO.CUSTOM_DVE_SPECS["ANT_MAGSQ"] = spec
    return op


def _build(gt: float, groups=None):
    import concourse.mybir as mybir
    import concourse.tile as tile
    from concourse import bacc

    f32 = mybir.dt.float32
    bf16 = mybir.dt.bfloat16
    AF = mybir.ActivationFunctionType
    magsq = _register_magsq()

    nc = bacc.Bacc("TRN2", target_bir_lowering=False, num_devices=NCORES)
    if groups is None:
        groups = [[0, 1, 2, 3], [4, 5, 6, 7]]

    xr_d = nc.dram_tensor("xr", [2, 8, 128, 512], bf16, kind="ExternalInput")
    xi_d = nc.dram_tensor("xi", [2, 8, 128, 512], bf16, kind="ExternalInput")
    wqk_d = {
        nm: nc.dram_tensor(nm, [128, 2, 8, 128], bf16, kind="ExternalInput")
        for nm in ("wqr", "wqi", "wkr", "wki")
    }
    wv_d = {
        nm: nc.dram_tensor(nm, [128, 8, 256], bf16, kind="ExternalInput")
        for nm in ("wvr", "wvi")
    }
    wo_d = {
        nm: nc.dram_tensor(nm, [128, 4, 2, 256], bf16, kind="ExternalInput")
        for nm in ("wor", "woi")
    }
    bqk_d = nc.dram_tensor("bqk", [128, 2, 4], f32, kind="ExternalInput")
    bo_d = nc.dram_tensor("bo", [128, 2, 2], f32, kind="ExternalInput")
    cos_d = nc.dram_tensor("cosd", [128, 1024], bf16, kind="ExternalInput")
    sin_d = nc.dram_tensor("sind", [128, 1024], bf16, kind="ExternalInput")
    o_r = nc.dram_tensor("o_r", [ODC, S], f32, kind="ExternalOutput")
    o_i = nc.dram_tensor("o_i", [ODC, S], f32, kind="ExternalOutput")

    rsem = [nc.alloc_semaphore("rsem0"), nc.alloc_semaphore("rsem1")]
    lsem = nc.alloc_semaphore("lsem")

    def mm(out, lhsT, rhs, start, stop):
        nc.tensor.matmul(out, lhsT=lhsT, rhs=rhs, start=start, stop=stop)

    with tile.TileContext(nc) as tc:
        with (
            tc.tile_pool(name="consts", bufs=1) as consts,
            tc.tile_pool(name="persist", bufs=1) as persist,
            tc.tile_pool(name="xp", bufs=16) as xp,
            tc.tile_pool(name="ptmp", bufs=3) as ptmp,
            tc.tile_pool(name="cp", bufs=4) as cp,
            tc.tile_pool(name="sqp", bufs=17) as sqp,
            tc.tile_pool(name="etp", bufs=17) as etp,
            tc.tile_pool(name="rp", bufs=2) as rp,
            tc.tile_pool(name="op", bufs=2) as op,
            tc.tile_pool(name="ps", bufs=8, space="PSUM") as ps,
            tc.tile_pool(name="dram", bufs=1, space="DRAM") as dram,
        ):
            # ---- persistent SBUF state ----
            Q = persist.tile([128, GH, S], bf16, tag="Q")
            K1 = persist.tile([128, GH, S], bf16, tag="K1")
            K2 = persist.tile([128, GH, S], bf16, tag="K2")
            Vr = persist.tile([128, 8, GH, 65], bf16, tag="Vr")
            Vi = persist.tile([128, 8, GH, 64], bf16, tag="Vi")
            recv = [
                persist.tile([128, 4, 4, 512], bf16, tag=f"recv{qc}",
                             name=f"recv{qc}")
                for qc in range(2)
            ]

            # ---- init: sem clear + cross-core barrier (re-exec safety) ----
            nc.gpsimd.sem_clear(rsem[0])
            nc.gpsimd.sem_clear(rsem[1])
            nc.gpsimd.sem_clear(lsem)
            bar_in = dram.tile([1, 4], f32, tag="bar_in", name="bar_in")
            bar_out = dram.tile([1, 4], f32, tag="bar_out", name="bar_out")
            barsb = consts.tile([1, 4], f32, tag="barsb")
            nc.vector.memset(barsb, 1.0)
            nc.sync.dma_start(bar_in, barsb)
            nc.gpsimd.collective_compute(
                "AllReduce",
                mybir.AluOpType.add,
                replica_groups=groups,
                ins=[bar_in[:].opt()],
                outs=[bar_out[:].opt()],
            )
            # data edges: gate every send payload chunk on barrier completion
            for qc in range(2):
                for ch in range(4):
                    nc.sync.dma_start(
                        recv[qc][0:1, 0, ch, 0:8].bitcast(f32), bar_out
                    )

            # ---- constants / weights (loaded once) ----
            nc.vector.memset(Vr[:, :, :, 64:65], 1.0)
            eps_t = consts.tile([128, 1], f32, tag="eps")
            nc.vector.memset(eps_t, 1e-6 * float(gt) * float(gt))
            cos_sb = consts.tile([128, 1024], bf16, tag="cos")
            nc.sync.dma_start(cos_sb, cos_d[:, :])
            sin_sb = consts.tile([128, 1024], bf16, tag="sin")
            nc.sync.dma_start(sin_sb, sin_d[:, :])
            bqk_sb = {}
            for pair in range(2):
                for kind in range(4):
                    t = consts.tile([128, 1], f32, tag=f"bqk{pair}{kind}",
                                    name=f"bqk{pair}{kind}")
                    nc.sync.dma_start(t, bqk_d[:, pair:pair + 1, kind])
                    bqk_sb[(pair, kind)] = t
            bo_sb = {}
            for ri in range(2):
                for odt in range(2):
                    t = consts.tile([128, 1], f32, tag=f"bo{ri}{odt}",
                                    name=f"bo{ri}{odt}")
                    nc.sync.dma_start(t, bo_d[:, ri:ri + 1, odt])
                    bo_sb[(ri, odt)] = t
            wqk_sb = {}
            for nm in wqk_d:
                t = consts.tile([128, 2, 8, 128], bf16, tag=nm)
                nc.sync.dma_start(t, wqk_d[nm][:, :, :, :])
                wqk_sb[nm] = t
            wv_sb = {}
            for nm in wv_d:
                t = consts.tile([128, 8, 256], bf16, tag=nm)
                nc.sync.dma_start(t, wv_d[nm][:, :, :])
                wv_sb[nm] = t
            wo_sb = {}
            for nm in wo_d:
                t = consts.tile([128, 4, 2, 256], bf16, tag=nm)
                nc.sync.dma_start(t, wo_d[nm][:, :, :, :])
                wo_sb[nm] = t

            def proj_pair(pair, w_sb, bias_kind, x_tiles, dst, half, csl):
                """One [128,512] PSUM pair-tile -> rope -> dst rows."""
                pst = ps.tile([128, 512], f32, tag="ps")
                for kt in range(8):
                    mm(pst, w_sb[:, pair, kt, :], x_tiles[kt],
                       start=(kt == 0), stop=(kt == 7))
                tb = ptmp.tile([128, 512], bf16, tag="tb")
                nc.scalar.activation(
                    tb, pst, AF.Identity, bias=bqk_sb[(pair, bias_kind)]
                )
                sh = ptmp.tile([128, 512], bf16, tag="sh")
                nc.vector.stream_shuffle(sh, tb, mask=_PAIRSWAP)
                nc.vector.tensor_mul(sh, sh, sin_sb[:, csl])
                t2 = ptmp.tile([128, 512], bf16, tag="t2")
                nc.vector.tensor_mul(t2, tb, cos_sb[:, csl])
                h0, h1 = 2 * pair, 2 * pair + 1
                r0 = half * 64
                nc.vector.tensor_add(
                    dst[r0:r0 + 64, h0, csl], t2[0:64, :], sh[0:64, :]
                )
                nc.vector.tensor_add(
                    dst[r0:r0 + 64, h1, csl], t2[64:128, :], sh[64:128, :]
                )

            def proj_c(c):
                csl = slice(c * 512, (c + 1) * 512)
                xr_t = []
                xi_t = []
                for kt in range(8):
                    t = xp.tile([128, 512], bf16, tag="xt")
                    nc.sync.dma_start(t, xr_d[c, kt, :, :])
                    xr_t.append(t)
                for kt in range(8):
                    t = xp.tile([128, 512], bf16, tag="xt")
                    nc.sync.dma_start(t, xi_d[c, kt, :, :])
                    xi_t.append(t)
                for pair in range(2):
                    proj_pair(pair, wqk_sb["wqr"], 0, xr_t, Q, 0, csl)
                    proj_pair(pair, wqk_sb["wqi"], 1, xi_t, Q, 1, csl)
                    proj_pair(pair, wqk_sb["wkr"], 2, xr_t, K1, 0, csl)
                    proj_pair(pair, wqk_sb["wki"], 3, xi_t, K1, 1, csl)
                    for h in (2 * pair, 2 * pair + 1):
                        nc.vector.tensor_scalar_mul(
                            K2[0:64, h, csl], K1[64:128, h, csl], -1.0
                        )
                        nc.vector.tensor_copy(K2[64:128, h, csl], K1[0:64, h, csl])
                # V projections: out [tok, vdim] (x as stationary)
                for w_sb, Vt, x_t, w65 in (
                    (wv_sb["wvr"], Vr, xr_t, True),
                    (wv_sb["wvi"], Vi, xi_t, False),
                ):
                    for tl in range(4):
                        tt = c * 4 + tl
                        pv = ps.tile([128, 256], f32, tag="ps")
                        for kt in range(8):
                            mm(pv, x_t[kt][:, tl * 128:(tl + 1) * 128],
                               w_sb[:, kt, :], start=(kt == 0), stop=(kt == 7))
                        dst = Vt[:, tt, :, 0:64] if w65 else Vt[:, tt, :, :]
                        nc.scalar.activation(dst, pv, AF.Identity)

            def qrange(qc, kvt):
                off = kvt - qc * 4
                qlo = 128 * off if off > 0 else 0
                return off, qlo

            def attn_qc(qc):
                nkv = 4 * (qc + 1)
                qbase = qc * 512
                for pair in range(2):
                    heads = (2 * pair, 2 * pair + 1)
                    tiles = []  # (h, kvt, qlo, sq)
                    for h in heads:
                        for kvt in range(nkv):
                            off, qlo = qrange(qc, kvt)
                            qs = slice(qbase + qlo, qbase + 512)
                            ksl = slice(kvt * 128, (kvt + 1) * 128)
                            psr = ps.tile([128, 512], f32, tag="ps")
                            mm(psr[:, qlo:], K1[:, h, ksl], Q[:, h, qs],
                               start=True, stop=True)
                            psi = ps.tile([128, 512], f32, tag="ps")
                            mm(psi[:, qlo:], K2[:, h, ksl], Q[:, h, qs],
                               start=True, stop=True)
                            c1 = cp.tile([128, 512], bf16, tag="c1")
                            nc.vector.tensor_copy(c1[:, qlo:], psi[:, qlo:])
                            sq = sqp.tile([128, 512], bf16, tag="sq")
                            nc.vector._custom_dve(
                                magsq, out=sq[:, qlo:], in0=psr[:, qlo:],
                                in1=c1[:, qlo:], imm2=float(gt) * float(gt),
                            )
                            tiles.append((h, kvt, qlo, sq))
                    # batched ACT passes (one table load per function)
                    for h, kvt, qlo, sq in tiles:
                        nc.scalar.activation(
                            sq[:, qlo:], sq[:, qlo:], AF.Sqrt, bias=eps_t
                        )
                    ets = {}
                    for h, kvt, qlo, sq in tiles:
                        et = etp.tile([128, 512], bf16, tag="et")
                        nc.scalar.activation(et[:, qlo:], sq[:, qlo:], AF.Exp)
                        ets[(h, kvt)] = et
                    for h, kvt, qlo, sq in tiles:
                        off = kvt - qc * 4
                        if off >= 0:
                            et = ets[(h, kvt)]
                            nc.gpsimd.affine_select(
                                out=et[:, qlo:],
                                in_=et[:, qlo:],
                                compare_op=mybir.AluOpType.is_ge,
                                fill=0.0,
                                base=0,
                                channel_multiplier=-1,
                                pattern=[[1, 512 - qlo]],
                            )
                    # AV + normalize per head
                    for sub, h in enumerate(heads):
                        avr = ps.tile([65, 512], f32, tag="ps")
                        avi = ps.tile([64, 512], f32, tag="ps")
                        for kvt in range(nkv):
                            off, qlo = qrange(qc, kvt)
                            et = ets[(h, kvt)]
                            mm(avr[:, qlo:], Vr[:, kvt, h, :], et[:, qlo:],
                               start=(kvt == 0), stop=(kvt == nkv - 1))
                            mm(avi[:, qlo:], Vi[:, kvt, h, :], et[:, qlo:],
                               start=(kvt == 0), stop=(kvt == nkv - 1))
                        den1 = rp.tile([1, 512], f32, tag="den1")
                        nc.vector.tensor_copy(den1, avr[64:65, :])
                        denb = rp.tile([64, 512], f32, tag="denb")
                        nc.gpsimd.partition_broadcast(denb, den1, channels=64)
                        rec = rp.tile([64, 512], f32, tag="rec")
                        nc.vector.reciprocal_approx_fast(rec, denb)
                        r0 = sub * 64
                        nc.vector.tensor_mul(
                            recv[qc][r0:r0 + 64, 0, 2 * pair, :], avr[0:64, :], rec
                        )
                        nc.vector.tensor_mul(
                            recv[qc][r0:r0 + 64, 0, 2 * pair + 1, :], avi[0:64, :], rec
                        )
                    # send this pair's chunks to the 3 XOR peers
                    if _NO_SEND:
                        continue
                    for d in (1, 2, 3):
                        rdests = [None] * 8
                        rdests[d] = (0, d)
                        nc.gpsimd.remote_dma_broadcast(
                            recv[qc][:, d, 2 * pair:2 * pair + 2, :],
                            recv[qc][:, 0, 2 * pair:2 * pair + 2, :],
                            remote_sem=rsem[qc],
                            local_sem=lsem,
                            rdests=rdests,
                        )
                    nc.gpsimd.trigger_dma(count=None)

            def oproj_qc(qc):
                # all 4 slots x 4 chunks must have arrived: 2 pairs x 3 peers
                # x (16//8) sem incs per qc, cumulative across qc
                if not (_NO_WAIT or _NO_SEND):
                    # 2 pairs x 3 peers x (16//8) sem incs per qc
                    with tc.tile_critical():
                        nc.scalar.wait_ge(rsem[qc], 12)
                        for s in (1, 2, 3):
                            nc.scalar.copy(
                                recv[qc][:, s, :, :], recv[qc][:, s, :, :]
                            )
                po = {}
                for ri in range(2):
                    for odt in range(2):
                        po[(ri, odt)] = ps.tile(
                            [128, 512], f32, tag="ps", name=f"po{ri}{odt}_{qc}"
                        )
                for s in range(4):
                    for ch in range(4):
                        pair, ri = ch // 2, ch % 2
                        w = wo_sb["wor" if ri == 0 else "woi"]
                        for odt in range(2):
                            mm(po[(ri, odt)],
                               w[:, s, pair, odt * 128:(odt + 1) * 128],
                               recv[qc][:, s, ch, :],
                               start=(s == 0 and pair == 0),
                               stop=(s == 3 and pair == 1))
                for ri, odst in ((0, o_r), (1, o_i)):
                    for odt in range(2):
                        oo = op.tile([128, 512], f32, tag="oo")
                        nc.scalar.activation(
                            oo, po[(ri, odt)], AF.Identity, bias=bo_sb[(ri, odt)]
                        )
                        nc.sync.dma_start(
                            odst[odt * 128:(odt + 1) * 128,
                                 qc * 512:(qc + 1) * 512],
                            oo,
                        )

            proj_c(0)
            attn_qc(0)
            proj_c(1)
            attn_qc(1)
            oproj_qc(0)
            oproj_qc(1)

    return nc


def _host_prep(inputs):
    """Fold ent/scale/bv on host; build per-core input maps in device layouts."""
    import ml_dtypes

    bf16 = ml_dtypes.bfloat16
    real = np.asarray(inputs["real"], np.float32)
    imag = np.asarray(inputs["imag"], np.float32)
    ent = np.asarray(inputs["ent"], np.float64)
    scale = 1.0 / math.sqrt(HD)

    def fold_w(W, do_ent, sc=1.0):
        W = np.asarray(W, np.float64).reshape(DIM, HEADS, HD)
        if do_ent:
            W = np.einsum("chd,hx->cxd", W, ent)
        return W * sc  # [DIM, HEADS, HD]

    def fold_b(b, do_ent, sc=1.0):
        b = np.asarray(b, np.float64).reshape(HEADS, HD)
        if do_ent:
            b = np.einsum("hd,hx->xd", b, ent)
        return b * sc

    Wq_r = fold_w(inputs["Wq_r"], True, scale)
    Wq_i = fold_w(inputs["Wq_i"], True, scale)
    Wk_r = fold_w(inputs["Wk_r"], True)
    Wk_i = fold_w(inputs["Wk_i"], True)
    Wv_r = fold_w(inputs["Wv_r"], False)
    Wv_i = fold_w(inputs["Wv_i"], False)
    bq_r = fold_b(inputs["bq_r"], True, scale)
    bq_i = fold_b(inputs["bq_i"], True, scale)
    bk_r = fold_b(inputs["bk_r"], True)
    bk_i = fold_b(inputs["bk_i"], True)
    Wo_r = np.asarray(inputs["Wo_r"], np.float64)
    Wo_i = np.asarray(inputs["Wo_i"], np.float64)
    bo_r = np.asarray(inputs["bo_r"], np.float64) + np.asarray(
        inputs["bv_r"], np.float64
    ) @ Wo_r
    bo_i = np.asarray(inputs["bo_i"], np.float64) + np.asarray(
        inputs["bv_i"], np.float64
    ) @ Wo_i

    strength = float(np.asarray(inputs["strength"]).reshape(-1)[0])
    temp = float(np.asarray(inputs["temp"]).reshape(-1)[0])
    gt = (1.0 / (1.0 + math.exp(-strength))) / max(temp, 0.01)

    # rope tables in pair-tile layout: row r (r%64 = d within head's 64 dims)
    rot_freqs = np.asarray(inputs["rot_freqs"], np.float64)  # [16]
    pos = np.arange(S, dtype=np.float64)
    emb = pos[:, None] * rot_freqs[None, :]  # [S, 16]
    cos_t = np.cos(emb)
    sin_t = np.sin(emb)
    cosd = np.ones((128, S), np.float64)
    sind = np.zeros((128, S), np.float64)
    for half in range(2):
        for d in range(ROTD):
            r = half * 64 + d
            cosd[r] = cos_t[:, d // 2]
            sind[r] = (-sin_t if d % 2 == 0 else sin_t)[:, d // 2]

    def pack_qk(Wf, g):
        # -> [128, 2, 8, 128]: [part, pair, kt, col]; col = (j//64)'th head of
        # pair, dim j%64 (dims 0..31 rot, 32..63 nr in natural order)
        Wc = Wf[:, 4 * g:4 * g + 4, :]  # [DIM, 4, 64]
        arr = Wc.reshape(8, 128, 2, 2, 64)  # [kt, part, pair, sub, d]
        arr = arr.transpose(1, 2, 0, 3, 4).reshape(128, 2, 8, 128)
        return np.ascontiguousarray(arr).astype(bf16)

    def pack_bqk_col(bf, g):
        # -> [128] rows: [h_even 64 dims | h_odd 64], per pair
        bc = bf[4 * g:4 * g + 4, :]  # [4, 64]
        return bc.reshape(2, 128)  # [pair, 128]

    in_maps = []
    for core in range(NCORES):
        b, g = core // 4, core % 4
        hs = slice(4 * g, 4 * g + 4)

        xT_r = real[b].T.astype(np.float64)  # [DIM, S]
        xT_i = imag[b].T.astype(np.float64)
        xr = xT_r.reshape(8, 128, 2, 512).transpose(2, 0, 1, 3)
        xi = xT_i.reshape(8, 128, 2, 512).transpose(2, 0, 1, 3)

        bqk = np.zeros((128, 2, 4), np.float32)
        for kind, bf in enumerate((bq_r, bq_i, bk_r, bk_i)):
            pc = pack_bqk_col(bf, g)  # [pair, 128]
            bqk[:, :, kind] = pc.T

        wv_pack = {}
        for nm, Wf in (("wvr", Wv_r), ("wvi", Wv_i)):
            Wc = Wf[:, hs, :].reshape(DIM, 256)  # [DIM, 4*64]
            arr = Wc.reshape(8, 128, 256)
            wv_pack[nm] = np.ascontiguousarray(arr.transpose(1, 0, 2)).astype(bf16)

        wo_pack = {}
        for nm, Wf in (("wor", Wo_r), ("woi", Wo_i)):
            arr = np.zeros((128, 4, 2, 256), np.float64)
            for s_ in range(4):
                gp = g ^ s_
                for pair in range(2):
                    for sub in range(2):
                        h = 4 * gp + 2 * pair + sub
                        arr[sub * 64:(sub + 1) * 64, s_, pair, :] = Wf[
                            h * 64:(h + 1) * 64, g * ODC:(g + 1) * ODC
                        ]
            wo_pack[nm] = np.ascontiguousarray(arr).astype(bf16)

        bo = np.zeros((128, 2, 2), np.float32)
        for ri, bv in enumerate((bo_r, bo_i)):
            for odt in range(2):
                bo[:, ri, odt] = bv[g * ODC + odt * 128: g * ODC + (odt + 1) * 128]

        m = {
            "xr": xr.astype(bf16),
            "xi": xi.astype(bf16),
            "wqr": pack_qk(Wq_r, g),
            "wqi": pack_qk(Wq_i, g),
            "wkr": pack_qk(Wk_r, g),
            "wki": pack_qk(Wk_i, g),
            "wvr": wv_pack["wvr"],
            "wvi": wv_pack["wvi"],
            "wor": wo_pack["wor"],
            "woi": wo_pack["woi"],
            "bqk": bqk,
            "bo": bo,
            "cosd": cosd.astype(bf16),
            "sind": sind.astype(bf16),
        }
        in_maps.append(m)
    return in_maps, gt


def kernel(**inputs):
    from concourse import bass_utils

    in_maps, gt = _host_prep(inputs)
    nc = _build(gt)
    nc.finalize()
    res = bass_utils.run_bass_kernel_spmd(nc, in_maps, core_ids=list(range(NCORES)))
    out_r = np.empty((B, S, DIM), np.float32)
    out_i = np.empty((B, S, DIM), np.float32)
    for core in range(NCORES):
        b, g = core // 4, core % 4
        out_r[b, :, g * ODC:(g + 1) * ODC] = res.results[core]["o_r"].T
        out_i[b, :, g * ODC:(g + 1) * ODC] = res.results[core]["o_i"].T
    return np.stack([out_r, out_i], axis=0)
